# revision 13
# baseline (speedup 1.0000x reference)
"""Causal self-attention on 8 Trainium2 NeuronCores.

Sharding: tensor-parallel on heads. Each core owns 2 of the 16 heads
(128 of the 1024 feature dims), computes QKV projections for its heads,
full causal attention for its heads over all 4 batch elements, and a
row-parallel partial of the output projection. The 8 partial outputs
(fp16) are summed on the host.

Layout strategy (everything contraction-dim-on-partitions):
  - x fed transposed: xT [C, B*T]
  - qT, kT computed as [hd, t] (hd = 2*64 local head dims stacked)
  - ST tile = S^T = k @ q^T in [t_k, t_q] layout, so softmaxed P^T is
    directly the rhs of the PV matmul (no transposes in the hot loop)
  - softmax denominator via an appended ones-column in the PV lhsT
  - 1/sum via reciprocal_approx_fast on a [2,512] row pair, broadcast
    across partitions with one K=2 selector matmul per q-block
  - matmul data in fp16 (full PE rate, 2^-11 rel err); softmax
    denominators kept in f32; exp biased by -2 so fp16 never overflows
    (bias cancels exactly in softmax)

Scheduling:
  - phase 1 streams x with 10 outstanding [128,2,512] DMA tiles so the
    PE is never DMA-starved
  - attention inner loop is software-pipelined: PV(kc) is emitted after
    ST(kc+2), hiding the scalar-engine exp latency
  - ST/exp restricted to the unmasked columns of diagonal chunks
  - normalize+project for q-block i runs interleaved into the chunk
    stream of q-block i+1, so the reciprocal chain never stalls the PE
"""

import json

import numpy as np

import concourse.bass as bass
import concourse.mybir as mybir
import concourse.tile as tile
import concourse.bass2jax as bass2jax
import concourse.bass_utils as bass_utils
from concourse.bass import ts
from concourse.masks import make_identity, make_upper_triangular

B, T, C, H, D = 4, 2048, 1024, 16, 64
NCORES = 8
HL = H // NCORES          # heads per core = 2
HD = HL * D               # local head dims = 128
TF = B * T                # flattened tokens = 8192
NKC = C // 128            # contraction chunks for projections = 8
NTB = TF // 512           # 512-wide token blocks = 16
QB = 512                  # q block width
NQB = T // QB             # q blocks per batch elem = 4
TKC = T // 128            # 128-wide k chunks per batch elem = 16

f32 = mybir.dt.float32
f16 = mybir.dt.float16
EXP = mybir.ActivationFunctionType.Exp
EXP_BIAS = -2.0           # exp(s - 2): keeps exp outputs well inside fp16

NP16 = np.float16


# --- workaround: this walrus build accepts at most one sync wait per
# instruction; Tile's final drain carries one wait per outstanding proc.
# Hoist surplus waits onto single-wait drain carriers in the BIR json.
_orig_compile_bir_kernel = None

MAX_WAITS_COMPUTE = 1
MAX_WAITS_CTRL = 1


def _split_waits_in_bir(bir_json):
    d = json.loads(bir_json)
    n = 0
    for f in d.get("functions", []):
        for bb in f.get("blocks", []):
            insts = bb.get("instructions", [])
            new_insts = []
            for inst in insts:
                si = inst.get("sync_info") or {}
                waits = si.get("on_wait") or []
                limit = (
                    MAX_WAITS_CTRL
                    if inst["opcode"]
                    in ("Drain", "EventSemaphore", "NoOp", "DMACopy", "DMA")
                    else MAX_WAITS_COMPUTE
                )
                if len(waits) > limit:
                    surplus = waits[:-limit]
                    for k, w in enumerate(surplus):
                        new_insts.append({
                            "name": f"{inst['name']}_wsplit{k}",
                            "engine": inst["engine"],
                            "opcode": "EventSemaphore",
                            "ins": [],
                            "outs": [],
                            "debug": inst.get("debug", 0),
                            "sync_info": {"on_update": [], "on_wait": [w]},
                        })
                        n += 1
                    si["on_wait"] = waits[-limit:]
                    inst["sync_info"] = si
                new_insts.append(inst)
            bb["instructions"] = new_insts
    return json.dumps(d).encode()


def _install_wait_split():
    global _orig_compile_bir_kernel
    if _orig_compile_bir_kernel is not None:
        return
    _orig_compile_bir_kernel = bass2jax.compile_bir_kernel

    def _patched(bir_json, tmpdir, neff_name="file.neff"):
        return _orig_compile_bir_kernel(
            _split_waits_in_bir(bir_json), tmpdir, neff_name
        )

    bass2jax.compile_bir_kernel = _patched


def build_program():
    nc = bass.Bass()
    xT = nc.declare_dram_parameter("xT", [C, TF], f16, isOutput=False)
    wqkvT = nc.declare_dram_parameter("wqkvT", [C, 3 * HD], f16, isOutput=False)
    wpT = nc.declare_dram_parameter("wpT", [HD, C], f16, isOutput=False)
    bqkv = nc.declare_dram_parameter("bqkv", [HD, 3], f32, isOutput=False)
    outT = nc.declare_dram_parameter("outT", [C, TF], f16, isOutput=True)

    with tile.TileContext(nc) as tc:
        with (
            tc.tile_pool(name="consts", bufs=1) as consts,
            tc.tile_pool(name="persist", bufs=1) as persist,
        ):
            ident = consts.tile([128, 128], f16)
            make_identity(nc, ident)
            tri = consts.tile([128, 128], f16)
            make_upper_triangular(nc, tri, val=1.0, diag=True)
            ones_f = consts.tile([128, HL], f16)
            nc.vector.memset(ones_f, 1.0)
            # selector for the denominator broadcast: rows 0 / 64 carry the
            # two heads' reciprocal rows (partition-aligned slots); the K=65
            # matmul replicates them across their 64-partition head ranges.
            # Unused rows are zero weights; rec65 keeps 1.0 there so the
            # products are exact zeros.
            sel65 = consts.tile([65, 128], f16)
            nc.vector.memset(sel65, 0.0)
            nc.vector.memset(sel65[0:1, 0:64], 1.0)
            nc.vector.memset(sel65[64:65, 64:128], 1.0)
            lnt = consts.tile([65, 512], f32)
            nc.vector.memset(lnt, 0.0)
            rec65 = consts.tile([65, 512], f16)
            nc.vector.memset(rec65, 1.0)
            expbias = consts.tile([128, 1], f32)
            nc.vector.memset(expbias, EXP_BIAS)

            wq_sb = consts.tile([128, NKC, 3 * HD], f16)
            wqr = wqkvT.rearrange("(kc p) n -> p kc n", p=128)
            for kc in range(NKC):
                # split per contraction chunk so the first QKV matmul only
                # waits on one eighth of the weight bytes
                nc.sync.dma_start(wq_sb[:, kc, :], wqr[:, kc, :])
            wp_sb = consts.tile([HD, C], f16)
            nc.sync.dma_start(wp_sb, wpT[:, :])
            b_sb = consts.tile([HD, 3], f32)
            nc.sync.dma_start(b_sb, bqkv[:, :])

            qT = persist.tile([128, TF], f16)
            kT = persist.tile([128, TF], f16)
            yT = persist.tile([128, TF], f16)
            # v in [t, hd] layout + a ones column per head for softmax sums
            v_sb = persist.tile([128, B, TKC, HL, 66], f16)
            for b_i in range(B):
                for kc_i in range(TKC):
                    nc.vector.tensor_copy(v_sb[:, b_i, kc_i, :, 64], ones_f)

            xTr = xT.rearrange("(kc p) t -> p kc t", p=128)

            # ---- phase 1: QKV projections (+ v transposed to [t, hd]) ----
            with (
                tc.tile_pool(name="p1", bufs=2) as p1,
                tc.tile_pool(name="ps1", bufs=4, space="PSUM") as ps1,
                tc.tile_pool(name="pst", bufs=2, space="PSUM") as pst,
            ):
                for tb in range(NTB):
                    tsl = ts(tb, 512)
                    psq = ps1.tile([128, 512], f32, tag="qkvps")
                    psk = ps1.tile([128, 512], f32, tag="qkvps")
                    psv = ps1.tile([128, 512], f32, tag="qkvps")
                    pss = [psq, psk, psv]
                    for kcp in range(NKC // 2):
                        # one DMA covers two contraction chunks: deeper
                        # prefetch + half the descriptor-issue cost
                        xt = p1.tile([128, 2, 512], f16, tag="xt", bufs=10)
                        nc.sync.dma_start(
                            xt, xTr[:, 2 * kcp:2 * kcp + 2, tsl]
                        )
                        for i in range(2):
                            kc = 2 * kcp + i
                            for pr in range(3):
                                nc.tensor.matmul(
                                    pss[pr],
                                    lhsT=wq_sb[:, kc, ts(pr, HD)],
                                    rhs=xt[:, i, :],
                                    start=(kc == 0),
                                    stop=(kc == NKC - 1),
                                )
                    # q/k moves on the scalar engine (idle in phase 1),
                    # v on DVE: balances the two engines
                    nc.scalar.add(qT[:, tsl], psq, b_sb[:, 0:1])
                    nc.scalar.add(kT[:, tsl], psk, b_sb[:, 1:2])
                    vt = p1.tile([128, 512], f16, tag="vt")
                    nc.vector.tensor_scalar_add(vt, psv, b_sb[:, 2:3])
                    for i in range(4):
                        tkidx = tb * 4 + i
                        b_i, kc_i = divmod(tkidx, TKC)
                        pt = pst.tile([128, 128], f16, tag="vtp")
                        nc.tensor.transpose(pt, vt[:, ts(i, 128)], ident)
                        nc.vector.tensor_copy(
                            v_sb[:, b_i, kc_i, :, 0:64],
                            pt[:, :].rearrange("p (h d) -> p h d", h=HL),
                        )

            # ---- phase 2: causal attention + output projection ----
            with (
                tc.tile_pool(name="p2", bufs=2) as p2,
                tc.tile_pool(name="ps2", bufs=1, space="PSUM") as ps2,
            ):
                def make_job(b_i, j):
                    q_off = b_i * T + j * QB
                    qsl = slice(q_off, q_off + QB)
                    nkc = 4 * (j + 1)
                    ypq = [
                        ps2.tile([65, 512], f32, tag=f"y{h}", bufs=1,
                                 name=f"ypq{h}")
                        for h in range(HL)
                    ]
                    exs = {}

                    def stem(kc):
                        # ST matmuls + exp + causal mask for one k-chunk
                        r = kc * 128 - j * QB
                        k_off = b_i * T + kc * 128
                        lo = max(r, 0)
                        st = ps2.tile([128, 2, 512], f32, tag="st", bufs=2,
                                      name="st")
                        for h in range(HL):
                            nc.tensor.matmul(
                                st[:, h, lo:512],
                                lhsT=kT[ts(h, 64), k_off:k_off + 128],
                                rhs=qT[ts(h, 64), q_off + lo:q_off + QB],
                                start=True,
                                stop=True,
                            )
                        ex = p2.tile([128, 2, 512], f16, tag="ex", bufs=3)
                        nc.scalar.activation(
                            ex[:, :, lo:512], st[:, :, lo:512], EXP,
                            scale=0.125, bias=expbias,
                        )
                        if r >= 0:
                            # causal mask on the gpsimd engine: keeps the
                            # DVE queue short so the per-job reciprocal
                            # chain never backs up behind mask ops
                            for h in range(HL):
                                nc.gpsimd.tensor_mul(
                                    ex[:, h, r:r + 128], ex[:, h, r:r + 128],
                                    tri,
                                )
                        exs[kc] = (ex, lo)

                    def pv(kc):
                        ex, lo = exs.pop(kc)
                        for h in range(HL):
                            nc.tensor.matmul(
                                ypq[h][:, lo:512],
                                lhsT=v_sb[:, b_i, kc, h, 0:65],
                                rhs=ex[:, h, lo:512],
                                start=(kc == 0),
                                stop=(kc == nkc - 1),
                            )

                    def fin_scalar():
                        # softmax denominators: 1/x = exp(-ln(x)) on the
                        # scalar engine, straight from the PSUM sums rows
                        # into partition-aligned slots 0/64.  Emitted right
                        # after the last PV so these run ahead of the next
                        # block's exps in the scalar queue.  lnt's padding
                        # rows stay 0 so rec65 padding is exp(0)=1 and the
                        # zero selector weights give exact zeros.
                        for h in range(HL):
                            nc.scalar.activation(
                                lnt[64 * h:64 * h + 1, :], ypq[h][64:65, :],
                                mybir.ActivationFunctionType.Ln,
                            )
                        nc.scalar.activation(rec65, lnt, EXP, scale=-1.0)

                    def fin_a():
                        # broadcast the reciprocal rows via selector matmul,
                        # stage to SBUF, then scale y into yT
                        bcrec = ps2.tile([128, 512], f32, tag="cp", bufs=2,
                                         name="bcrec")
                        nc.tensor.matmul(
                            bcrec, lhsT=sel65, rhs=rec65, start=True, stop=True
                        )
                        rb = p2.tile([64, 2, 512], f16, tag="rb", bufs=2)
                        for h in range(HL):
                            nc.vector.tensor_copy(
                                rb[:, h, :], bcrec[64 * h:64 * h + 64, :]
                            )
                        for h in range(HL):
                            nc.vector.tensor_mul(
                                yT[ts(h, 64), qsl], ypq[h][0:64, :],
                                rb[:, h, :],
                            )

                    def fin_b():
                        # row-parallel output projection, fp16 out,
                        # 2 output-row chunks per DMA
                        for op in range(4):
                            ob = p2.tile([128, 2, 512], f16, tag="ob", bufs=4)
                            for i in range(2):
                                oc = 2 * op + i
                                pp = ps2.tile([128, 512], f32, tag="cp",
                                              bufs=2, name="pp")
                                nc.tensor.matmul(
                                    pp,
                                    lhsT=wp_sb[:, ts(oc, 128)],
                                    rhs=yT[:, qsl],
                                    start=True,
                                    stop=True,
                                )
                                nc.vector.tensor_copy(ob[:, i, :], pp)
                            nc.sync.dma_start(
                                outT[op * 256:(op + 1) * 256, qsl].rearrange(
                                    "(c p) t -> p c t", p=128
                                ),
                                ob,
                            )

                    return nkc, stem, pv, fin_scalar, fin_a, fin_b

                # Emission order per job i (PE queue = program order):
                #   stems(i,0..1) | fin_a(i-1): sel+norm | pv/stem chunk loop
                #   | fin_scalar(i): ln/exp65 | fin_b(i-1): 8 proj matmuls.
                # The proj matmuls of job i-1 land right where job i's
                # reciprocal chain runs on the scalar engine, so the PE
                # always has ~2-3us of independent work during the chain.
                jobs = [(b_i, j) for b_i in range(B) for j in range(NQB)]
                prev_a = prev_b = None
                for b_i, j in jobs:
                    nkc, stem, pv, fin_scalar, fin_a, fin_b = make_job(b_i, j)
                    stem(0)
                    stem(1)
                    if prev_a is not None:
                        prev_a()
                    pv(0)
                    if nkc > 2:
                        stem(2)
                    for kc in range(1, nkc):
                        pv(kc)
                        if kc + 2 < nkc:
                            stem(kc + 2)
                    fin_scalar()
                    if prev_b is not None:
                        prev_b()
                    prev_a, prev_b = fin_a, fin_b
                prev_a()
                prev_b()
    return nc


_program = None


def _get_program():
    global _program
    if _program is None:
        _install_wait_split()
        _program = build_program()
    return _program


def kernel(x, Wq, bq, Wk, bk, Wv, bv, Wp, bp):
    nc = _get_program()

    x = np.asarray(x, dtype=np.float32)
    xT = np.ascontiguousarray(x.reshape(TF, C).T.astype(NP16))
    in_maps = []
    for core in range(NCORES):
        rows = slice(core * HD, (core + 1) * HD)
        wqkvT = np.ascontiguousarray(
            np.concatenate(
                [np.asarray(W, np.float32)[rows].T for W in (Wq, Wk, Wv)], axis=1
            ).astype(NP16)
        )
        wpT = np.ascontiguousarray(np.asarray(Wp, np.float32)[:, rows].T.astype(NP16))
        bq_l = np.stack(
            [np.asarray(v, np.float32)[rows] for v in (bq, bk, bv)], axis=1
        )
        in_maps.append(
            {
                "xT": xT,
                "wqkvT": wqkvT,
                "wpT": wpT,
                "bqkv": np.ascontiguousarray(bq_l),
            }
        )

    r = bass_utils.run_bass_kernel_spmd(nc, in_maps, list(range(NCORES)))
    acc = r.results[0]["outT"].astype(np.float32)
    for core in range(1, NCORES):
        acc = acc + r.results[core]["outT"].astype(np.float32)
    out = acc.T.reshape(B, T, C) + np.asarray(bp, np.float32)[None, None, :]
    return out.astype(np.float32)
